# revision 1
# baseline (speedup 1.0000x reference)
"""Causal self-attention on 8 trn2 NeuronCores — fp8-DR scores + spliced
emission schedule.

Sharding: tensor-parallel over heads (2 heads/core).

Key techniques vs baseline:
  - Score matmuls in fp8e4 DoubleRow (0.5 cyc/row): lhsT kT8 carries an
    [k_hi, k_lo] e4m3 residual pair in the DR slots (k at ~7 mantissa
    bits), rhs is qT8 e4m3 stride-0-broadcast into both slots.
  - Optional QK_PROJ_FP8: q/k projections fp8 DR (x e4m3 moving, W as
    32-scaled hi/lo e4m3 pair in the slots).
  - v_aug per head = [1 | 0*63 | v]: PSUM row 0 of the PV accumulation
    is the softmax denominator (reciprocal needs partition 0); rows
    64.. are attn dims (64-aligned for the normalize multiply).
  - v_aug assembled by DMA-transpose (frees DVE + PE transpose).
  - Normalization direct from PSUM: recip(row 0) -> partition_broadcast
    -> tensor_mul into bf16 at_bj.
  - Emission-order splicing: proj/out-proj work is chopped into small
    units and interleaved between score and PV matmuls so the in-order
    PE queue always has filler while exp (Act) catches up; Act runs
    exp exclusively.
  - Band-limited causal affine_select (128 cols instead of row tail).
"""

import sys

if "/opt/trn_rl_repo" not in sys.path:
    sys.path.insert(0, "/opt/trn_rl_repo")

from collections import deque

import numpy as np
import ml_dtypes

import concourse.bass as bass
import concourse.tile as tile
from concourse import bacc, mybir
from concourse.bass_utils import run_bass_kernel_spmd
from concourse.masks import make_identity

BF16 = mybir.dt.bfloat16
F8 = mybir.dt.float8e4
F32 = mybir.dt.float32
AF = mybir.ActivationFunctionType
DR = mybir.MatmulPerfMode.DoubleRow

N_EMBED = 1024
N_HEAD = 16
HEAD_DIM = 64
N_CORES = 8
HEADS_PER_CORE = N_HEAD // N_CORES          # 2
DCORE = HEADS_PER_CORE * HEAD_DIM           # 128
B = 2
S = 2048
QB = 512
KT = 128
DT = N_EMBED // 128                         # 8
SCALE = 1.0 / 8.0
HB = 128                                    # v_aug per-head block width
H = HEADS_PER_CORE

QK_PROJ_FP8 = False          # phase 2 switch
WSCALE = 32.0
YSB_ACT = 0                  # of 8 ysb copies per block, how many on Act


def build_program(seq=S):
    s_tot = B * seq
    n_qb = seq // QB
    n_kt = seq // KT
    kt_per_qb = QB // KT

    qk_scale = SCALE / (WSCALE * WSCALE) if QK_PROJ_FP8 else SCALE

    nc = bacc.Bacc("TRN2", target_bir_lowering=False, debug=False,
                   num_devices=N_CORES)

    xT = nc.dram_tensor("xT", [N_EMBED, s_tot], BF16, kind="ExternalInput")
    wv = nc.dram_tensor("wv", [N_EMBED, DCORE], BF16, kind="ExternalInput")
    bq = nc.dram_tensor("bq", [DCORE, 1], F32, kind="ExternalInput")
    bk = nc.dram_tensor("bk", [DCORE, 1], F32, kind="ExternalInput")
    wout = nc.dram_tensor("wout", [DCORE, N_EMBED], BF16, kind="ExternalInput")
    y = nc.dram_tensor("y", [s_tot, N_EMBED], BF16, kind="ExternalOutput")
    if QK_PROJ_FP8:
        wq = nc.dram_tensor("wq", [N_EMBED, 2, DCORE], F8, kind="ExternalInput")
        wk = nc.dram_tensor("wk", [N_EMBED, 2, DCORE], F8, kind="ExternalInput")
    else:
        wq = nc.dram_tensor("wq", [N_EMBED, DCORE], BF16, kind="ExternalInput")
        wk = nc.dram_tensor("wk", [N_EMBED, DCORE], BF16, kind="ExternalInput")

    xT_r = xT.ap().rearrange("(t p) s -> p t s", p=128)

    with (
        tile.TileContext(nc) as tc,
        tc.tile_pool(name="singles", bufs=1) as singles,
        # PSUM banks (8): s_ps 2x2 = 4, attn 2x1 = 2, aux 2x1 = 2
        tc.tile_pool(name="s_ps", bufs=2, space="PSUM") as s_pool,
        tc.tile_pool(name="attn_ps", bufs=1, space="PSUM") as attn_pool,
        tc.tile_pool(name="aux_ps", bufs=2, space="PSUM") as aux_pool,
        tc.tile_pool(name="vstage", bufs=3) as vstage_pool,
        tc.tile_pool(name="pt_sb", bufs=10) as pt_pool,
        tc.tile_pool(name="rec_sb", bufs=3) as rec_pool,
        tc.tile_pool(name="bc_sb", bufs=3) as bc_pool,
        tc.tile_pool(name="at_sb", bufs=4) as at_pool,
        tc.tile_pool(name="y_sb", bufs=6) as ysb_pool,
    ):
        # ---- persistent SBUF tensors ----
        xT_sb = singles.tile([128, DT, s_tot], BF16)
        if QK_PROJ_FP8:
            x8_sb = singles.tile([128, DT, s_tot], F8)
            wq_sb = singles.tile([128, DT, 2, DCORE], F8)
            wk_sb = singles.tile([128, DT, 2, DCORE], F8)
        else:
            wq_sb = singles.tile([128, DT, DCORE], BF16)
            wk_sb = singles.tile([128, DT, DCORE], BF16)
        wv_sb = singles.tile([128, DT, DCORE], BF16)
        bq_sb = singles.tile([DCORE, 1], F32)
        bk_sb = singles.tile([DCORE, 1], F32)
        wout_sb = singles.tile([DCORE, N_EMBED], BF16)
        qT8 = singles.tile([DCORE, s_tot], F8)
        kT8 = singles.tile([DCORE, 2, s_tot], F8)   # [k_hi | k_lo] DR slots
        v_aug = singles.tile([128, B * n_kt, 2 * HB], BF16)
        ident_sb = singles.tile([128, 128], BF16)

        # ---- input DMAs ----
        if QK_PROJ_FP8:
            nc.sync.dma_start(
                out=wq_sb, in_=wq.ap().rearrange("(t p) r h -> p t r h", p=128))
            nc.sync.dma_start(
                out=wk_sb, in_=wk.ap().rearrange("(t p) r h -> p t r h", p=128))
        else:
            nc.sync.dma_start(
                out=wq_sb, in_=wq.ap().rearrange("(t p) h -> p t h", p=128))
            nc.sync.dma_start(
                out=wk_sb, in_=wk.ap().rearrange("(t p) h -> p t h", p=128))
        nc.sync.dma_start(out=bq_sb, in_=bq.ap())
        nc.sync.dma_start(out=xT_sb[:, :, 0:QB], in_=xT_r[:, :, 0:QB])
        nc.sync.dma_start(out=wv_sb,
                          in_=wv.ap().rearrange("(t p) h -> p t h", p=128))
        nc.sync.dma_start(out=bk_sb, in_=bk.ap())
        nc.sync.dma_start(out=wout_sb, in_=wout.ap())
        for sb in range(1, s_tot // QB):
            sl = slice(sb * QB, (sb + 1) * QB)
            nc.sync.dma_start(out=xT_sb[:, :, sl], in_=xT_r[:, :, sl])

        make_identity(nc, ident_sb)
        # warm the PE during the input-DMA wait: continuous dummy matmuls
        # keep pe_busy_start early so real work starts at full clock
        warm_ps = aux_pool.tile([128, 128], F32, tag="aux", name="warm")
        for _ in range(60):
            nc.tensor.matmul(warm_ps, lhsT=ident_sb, rhs=ident_sb[:, 0:128],
                             start=True, stop=True)
        # v_aug per-head block: [1 | zeros(63) | v(64)] (pads on gpsimd)
        nc.gpsimd.memset(v_aug[:, :, 0:HEAD_DIM], 0.0)
        nc.gpsimd.memset(v_aug[:, :, HB:HB + HEAD_DIM], 0.0)
        nc.gpsimd.memset(v_aug[:, :, 0], 1.0)
        nc.gpsimd.memset(v_aug[:, :, HB], 1.0)

        def proj_mm(ps, w_sb, sl, t):
            if QK_PROJ_FP8 and w_sb is not wv_sb:
                nc.tensor.matmul(
                    ps, lhsT=w_sb[:, t],
                    rhs=x8_sb[:, t, sl].unsqueeze(1).broadcast_to(
                        [128, 2, QB]),
                    start=(t == 0), stop=(t == DT - 1), perf_mode=DR)
            else:
                nc.tensor.matmul(ps, lhsT=w_sb[:, t], rhs=xT_sb[:, t, sl],
                                 start=(t == 0), stop=(t == DT - 1))

        def proj_gen(sb):
            """Emission units for projections of row-block sb."""
            sl = slice(sb * QB, (sb + 1) * QB)
            if QK_PROJ_FP8:
                # on-device e4m3 cast of x for this block (idle Pool engine)
                for t in range(DT):
                    nc.gpsimd.tensor_copy(x8_sb[:, t, sl], xT_sb[:, t, sl])
                    if t % 3 == 2:
                        yield
                yield
            # --- q ---
            ps = aux_pool.tile([128, QB], F32, tag="aux", name="proj_q")
            for t in range(DT):
                proj_mm(ps, wq_sb, sl, t)
                if t == 3:
                    yield
            yield
            nc.vector.tensor_scalar_add(qT8[:, sl], ps, bq_sb)
            yield
            # --- k: hi + residual lo ---
            ps = aux_pool.tile([128, QB], F32, tag="aux", name="proj_k")
            for t in range(DT):
                proj_mm(ps, wk_sb, sl, t)
                if t == 3:
                    yield
            yield
            nc.vector.tensor_scalar_add(kT8[:, 0, sl], ps, bk_sb)
            nc.vector.scalar_tensor_tensor(
                out=kT8[:, 1, sl], in0=ps, scalar=bk_sb,
                op0=mybir.AluOpType.add, in1=kT8[:, 0, sl],
                op1=mybir.AluOpType.subtract)
            yield
            # --- v ---
            ps = aux_pool.tile([128, QB], F32, tag="aux", name="proj_v")
            for t in range(DT):
                proj_mm(ps, wv_sb, sl, t)
                if t == 3:
                    yield
            yield
            vstage = vstage_pool.tile([128, QB], BF16)
            nc.vector.tensor_copy(vstage, ps)
            yield
            for u in range(QB // 128):
                kt_gl = (QB // 128) * sb + u
                for h in range(H):
                    nc.sync.dma_start_transpose(
                        out=v_aug[:, kt_gl, HB * h + HEAD_DIM:HB * (h + 1)],
                        in_=vstage[HEAD_DIM * h:HEAD_DIM * (h + 1),
                                   u * 128:(u + 1) * 128])
                yield

        def outproj_gen(b_i, j, at_bj, act_share=YSB_ACT, tail=False):
            del tail
            """Emission units for the out-projection of q-block j."""
            for qt in range(QB // 128):
                at = at_bj[:, qt * 128:(qt + 1) * 128]
                ysb = ysb_pool.tile([128, N_EMBED], BF16, tag="ysb",
                                    name="ysb")
                for u in range(N_EMBED // QB):
                    yp = aux_pool.tile([128, QB], F32, tag="aux", name="yp")
                    nc.tensor.matmul(yp, lhsT=at,
                                     rhs=wout_sb[:, u * QB:(u + 1) * QB],
                                     start=True, stop=True)
                    dst = ysb[:, u * QB:(u + 1) * QB]
                    if qt * 2 + u < act_share:
                        nc.scalar.copy(dst, yp)
                    else:
                        nc.vector.tensor_copy(dst, yp)
                    yield
                row0 = b_i * seq + j * QB + qt * 128
                nc.sync.dma_start(out=y.ap()[row0:row0 + 128, :], in_=ysb)
                yield

        units = deque()          # outproj units
        punits = deque()         # proj units (ready earlier; prefer early)

        def _pump_from(q, n):
            while n > 0 and q:
                try:
                    next(q[0])
                    n -= 1
                except StopIteration:
                    q.popleft()
            return n

        def pump(n, prefer_proj=False):
            if prefer_proj:
                n = _pump_from(punits, n)
                _pump_from(units, n)
            else:
                n = _pump_from(units, n)
                _pump_from(punits, n)

        def attn_kloop(b_i, j, boost=False):
            """Scores/exp/mask/PV; diagonal k-tiles first; filler pumped
            between the score and (deferred) PV matmuls."""
            q0 = b_i * seq + j * QB
            attn_ps = [attn_pool.tile([HB, QB], F32, tag=f"attn{h}",
                                      name=f"attn{h}") for h in range(H)]
            kts = list(range(kt_per_qb * j, kt_per_qb * (j + 1))) + \
                list(range(0, kt_per_qb * j))

            def emit_pv(kt, pt, off, pos):
                for h in range(H):
                    nc.tensor.matmul(
                        attn_ps[h][:, off:],
                        lhsT=v_aug[:, b_i * n_kt + kt, HB * h:HB * (h + 1)],
                        rhs=pt[:, h, off:],
                        start=(pos == 0), stop=(pos == len(kts) - 1))

            pending = deque()    # PV deferred two k-tiles behind the scores
            for pos, kt in enumerate(kts):
                ks = slice(b_i * seq + kt * 128, b_i * seq + kt * 128 + 128)
                d = kt - kt_per_qb * j
                off = 128 * d if d >= 0 else 0
                cols = QB - off
                s_ps = s_pool.tile([128, H, QB], F32, tag="s", name="s_ps")
                pt = pt_pool.tile([128, H, QB], BF16, tag="pt", name="pt")
                for h in range(H):
                    hsl = slice(HEAD_DIM * h, HEAD_DIM * (h + 1))
                    nc.tensor.matmul(
                        s_ps[:, h, off:],
                        lhsT=kT8[hsl, :, ks],
                        rhs=qT8[hsl, q0 + off:q0 + QB].unsqueeze(1)
                            .broadcast_to([HEAD_DIM, 2, cols]),
                        start=True, stop=True, perf_mode=DR)
                nc.scalar.activation(pt[:, :, off:], s_ps[:, :, off:],
                                     AF.Exp, scale=qk_scale)
                if d >= 0:  # mask only the 128-col diagonal band
                    nc.gpsimd.affine_select(
                        out=pt[:, :, off:off + 128],
                        in_=pt[:, :, off:off + 128],
                        compare_op=mybir.AluOpType.is_ge, fill=0.0,
                        base=0, channel_multiplier=-1,
                        pattern=[[0, H], [1, 128]])
                pump(2, prefer_proj=(pos < 6))
                pending.append((kt, pt, off, pos))
                if len(pending) > 6:
                    emit_pv(*pending.popleft())
            while pending:
                emit_pv(*pending.popleft())
            # normalize straight from PSUM; row 0 is the denominator
            at_bj = at_pool.tile([DCORE, QB], BF16, name="at_bj")
            for h in range(H):
                rf = rec_pool.tile([1, QB], F32, tag=f"rf{h}", name=f"rf{h}")
                nc.vector.reciprocal_approx_fast(rf, attn_ps[h][0:1, :])
                bc_sb = bc_pool.tile([128, QB], F32, tag=f"bc{h}",
                                     name=f"bc{h}")
                nc.gpsimd.partition_broadcast(bc_sb, rf)
                nc.vector.tensor_mul(
                    at_bj[HEAD_DIM * h:HEAD_DIM * (h + 1), :],
                    attn_ps[h][HEAD_DIM:HB, :], bc_sb[HEAD_DIM:HB, :])
            return at_bj

        # ---- schedule ----
        n_blocks = B * n_qb
        next_proj = 0
        for _ in range(3):
            if next_proj < n_blocks:
                punits.append(proj_gen(next_proj))
                next_proj += 1
        _pump_from(punits, 10 ** 9)   # frontload fully
        for b_i in range(B):
            for j in range(n_qb):
                if next_proj < n_blocks:
                    punits.append(proj_gen(next_proj))
                    next_proj += 1
                at_bj = attn_kloop(b_i, j)
                tail = (b_i == B - 1 and j == n_qb - 1)
                units.append(outproj_gen(b_i, j, at_bj,
                                         act_share=4 if tail else YSB_ACT))
        pump(10 ** 9)
        _pump_from(punits, 10 ** 9)   # drain the tail

    nc.compile()
    return nc


_CACHE = {}


def _get_program(seq=S):
    if seq not in _CACHE:
        _CACHE[seq] = build_program(seq)
    return _CACHE[seq]


def make_in_maps(x, W_qkv, b_qkv, seq=S):
    bf16 = ml_dtypes.bfloat16
    e4 = ml_dtypes.float8_e4m3
    s_tot = B * seq
    xT = np.ascontiguousarray(x.reshape(s_tot, N_EMBED).T).astype(bf16)
    in_maps = []
    for c in range(N_CORES):
        csl = slice(DCORE * c, DCORE * (c + 1))
        m = {
            "xT": xT,
            "wv": np.ascontiguousarray(
                W_qkv[:, 2 * N_EMBED:][:, csl]).astype(bf16),
            "bq": np.ascontiguousarray(
                b_qkv[csl].reshape(DCORE, 1)).astype(np.float32),
            "bk": np.ascontiguousarray(
                b_qkv[N_EMBED:][csl].reshape(DCORE, 1)).astype(np.float32),
            "wout": None,  # filled by caller
        }
        if QK_PROJ_FP8:
            for nm, off in (("wq", 0), ("wk", N_EMBED)):
                w = (WSCALE * W_qkv[:, off:][:, csl]).astype(np.float32)
                w_hi = w.astype(e4)
                w_lo = (w - w_hi.astype(np.float32)).astype(e4)
                m[nm] = np.ascontiguousarray(np.stack([w_hi, w_lo], axis=1))
            m["bq"] = m["bq"] * WSCALE
            m["bk"] = m["bk"] * WSCALE
        else:
            m["wq"] = np.ascontiguousarray(W_qkv[:, csl]).astype(bf16)
            m["wk"] = np.ascontiguousarray(
                W_qkv[:, N_EMBED:][:, csl]).astype(bf16)
        in_maps.append(m)
    return in_maps


def kernel(x, W_qkv, b_qkv, W_out, b_out):
    x = np.asarray(x, dtype=np.float32)
    W_qkv = np.asarray(W_qkv, dtype=np.float32)
    b_qkv = np.asarray(b_qkv, dtype=np.float32)
    W_out = np.asarray(W_out, dtype=np.float32)
    b_out = np.asarray(b_out, dtype=np.float32)

    nc = _get_program(S)
    in_maps = make_in_maps(x, W_qkv, b_qkv, S)
    bf16 = ml_dtypes.bfloat16
    for c in range(N_CORES):
        csl = slice(DCORE * c, DCORE * (c + 1))
        in_maps[c]["wout"] = np.ascontiguousarray(W_out[csl, :]).astype(bf16)

    res = run_bass_kernel_spmd(nc, in_maps, core_ids=list(range(N_CORES)))
    y = np.zeros((B * S, N_EMBED), dtype=np.float32)
    for r in res.results:
        y += r["y"].astype(np.float32)
    y += b_out[None, :] + b_qkv[2 * N_EMBED:] @ W_out
    return y.reshape(B, S, N_EMBED)

